# revision 12
# baseline (speedup 1.0000x reference)
"""Bayesian linear layer (per-sample weights) on 8 Trainium2 NeuronCores.

out[b,o] = sum_i x[b,i] * (eps[b,i,o]*softplus(ro)[i,o] + mu[i,o])
           + eps_bias[b,o]*softplus(ro_bias)[o] + mu_bias[o]

Strategy (2D sharding: 2 batch-groups x 4 i-quarters per core):
  - Each core handles 64 samples and 256 of the 1024 contraction rows,
    producing a partial sum; the host unshard adds the four i-quarters.
    This quarters the replicated ro/mu traffic (HBM-domain bandwidth,
    ~410 GB/s per NC with the pair saturated, is the binding resource).
  - eps streams as SAMPLE PAIRS: 2MiB contiguous per transfer
    ([128, 2, 2048] tile) on the sync HWDGE ring — 1MiB transfers
    measured only ~330-370 GB/s effective; 2MiB reach ~410. ALL loads
    share that ring, sigma params ahead of the stream: SDMA round-robins
    between rings at whole-transfer granularity, so a param load on any
    other ring starves behind queued eps transfers.
  - Contraction rows are mapped p-major (i_local = p*CPP + c), so each
    partition's bytes are one 8KB contiguous run per sample (fewer,
    bigger DMA descriptors); the host mirrors this in the x layout.
  - DVE multiplies tiles by softplus(ro) (grouped Exp/Ln ACT passes),
    rounding to float32r so TensorE consumes them at 1 cycle/row.
  - TensorE reduces over i with M=1 matmuls (lhsT = x column) into a
    [1,1024] PSUM tile per sample; the scalar engine copies PSUM->SBUF.
    Output rows are BATCHED 4 samples per store (16KB on the gpsimd
    ring): per-row 4KB stores interrupt the SDMA engine serving
    partition 0 every sample and skew eps-transfer completions
    (measured ~+0.8us per 2MiB pair).
  - x@mu partials ship as a separate [64,1024] output; the host unshard
    adds them plus the (elementwise) bias row during the gather, so no
    per-sample bias matmuls are needed.
  - Tail: the final chunk of the last sample is DMA'd and TT'd in
    512-column halves so the matmul/evacuation chain starts before the
    last bytes land.
"""

import numpy as np

import concourse.bass as bass
import concourse.bacc as bacc
import concourse.mybir as mybir
from concourse.tile import TileContext
from concourse.bass_utils import run_bass_kernel_spmd

F32 = mybir.dt.float32
F32R = mybir.dt.float32r
AF = mybir.ActivationFunctionType

B, IN, OUT = 128, 1024, 1024
NCORES = 8
BG = 2                    # batch groups
ISH = NCORES // BG        # i-shards (4)
BS = B // BG              # 64 samples per core
INS = IN // ISH           # 256 contraction rows per core
P = 128
CPP = INS // P            # 2 contraction rows per partition
FREE = CPP * OUT          # 2048 free elems per sample
NPAIR = BS // 2           # samples stream in pairs (2MiB transfers)
OB = 4                    # output rows batched per store


def build_nc():
    nc = bacc.Bacc(None, target_bir_lowering=False)

    eps_d = nc.declare_dram_parameter("eps", [BS, INS, OUT], F32, isOutput=False)
    ro_d = nc.declare_dram_parameter("ro", [INS, OUT], F32, isOutput=False)
    mu_d = nc.declare_dram_parameter("mu", [INS, OUT], F32, isOutput=False)
    # xt[p, c*BS + b] = x[b, ishard*INS + p*CPP + c]  (host-side layout)
    xt_d = nc.declare_dram_parameter("xt", [P, CPP * BS], F32, isOutput=False)
    out_d = nc.declare_dram_parameter("out", [1, BS * OUT], F32, isOutput=True)
    xmu_d = nc.declare_dram_parameter("xmu", [BS, OUT], F32, isOutput=True)

    # i_local = p*CPP + c: p-major, 8KB per-partition contiguous runs
    ro_r = ro_d.rearrange("(p c) o -> p c o", p=P)
    mu_r = mu_d.rearrange("(p c) o -> p c o", p=P)

    with TileContext(nc) as tc:
        with (
            tc.tile_pool(name="const", bufs=1) as cpool,
            tc.tile_pool(name="eps", bufs=6) as epool,
            tc.tile_pool(name="epr", bufs=5) as eprpool,
            tc.tile_pool(name="small", bufs=3) as spool,
            tc.tile_pool(name="psmu", bufs=1, space="PSUM") as pmupool,
            tc.tile_pool(name="psum", bufs=3, space="PSUM") as ppool,
        ):
            # ---- sigma params first on the ring, then the eps stream ----
            sig = cpool.tile([P, FREE], F32)
            for h in range(CPP):
                nc.sync.dma_start(
                    out=sig[:, h * OUT : (h + 1) * OUT], in_=ro_r[:, h : h + 1, :]
                )
            xt = cpool.tile([P, CPP * BS], F32)
            nc.sync.dma_start(out=xt, in_=xt_d[:, :])
            for h in range(CPP):
                sl = sig[:, h * OUT : (h + 1) * OUT]
                nc.scalar.activation(sl, sl, AF.Exp)
            for h in range(CPP):
                sl = sig[:, h * OUT : (h + 1) * OUT]
                nc.scalar.activation(sl, sl, AF.Ln, bias=1.0)

            xtr = cpool.tile([P, CPP * BS], F32R)
            nc.vector.tensor_copy(out=xtr, in_=xt)

            # eps pair 0 ahead of mu: the big stream starts as early as
            # possible; mu is only needed by the x@mu partial.
            ep_first = epool.tile([P, 2 * FREE], F32, tag="ep")
            nc.sync.dma_start(
                out=ep_first,
                in_=eps_d[0:2, :, :].rearrange("t (p c) o -> p t (c o)", p=P),
            )

            # ---- x @ mu (partial over this core's i rows) ---------------
            psmu = pmupool.tile([BS, OUT], F32)
            mt = cpool.tile([P, FREE], F32)
            nc.sync.dma_start(out=mt, in_=mu_r[:, :, :])
            for c in range(CPP):
                for nh in range(2):
                    nc.tensor.matmul(
                        psmu[:, nh * 512 : (nh + 1) * 512],
                        xt[:, c * BS : (c + 1) * BS],
                        mt[:, c * OUT + nh * 512 : c * OUT + (nh + 1) * 512],
                        start=(c == 0),
                        stop=(c == CPP - 1),
                    )
            oxmu = spool.tile([BS, OUT], F32, tag="oxmu")
            nc.scalar.copy(oxmu, psmu[:, :])
            nc.scalar.dma_start(out=xmu_d[:, :], in_=oxmu)

            # ---- main streaming loop: one 2MiB DMA per sample pair ------
            obat = None
            for t in range(NPAIR):
                lastpair = t == NPAIR - 1
                if t == 0:
                    ep = ep_first
                else:
                    ep = epool.tile([P, 2 * FREE], F32, tag="ep")
                    if not lastpair:
                        nc.sync.dma_start(
                            out=ep,
                            in_=eps_d[2 * t : 2 * t + 2, :, :].rearrange(
                                "t (p c) o -> p t (c o)", p=P
                            ),
                        )
                    else:
                        # fine-grained tail: per-chunk DMAs, final chunk
                        # split by o-halves
                        for u in range(2):
                            src_b = eps_d[2 * t + u, :, :].rearrange(
                                "(p c) o -> p (c o)", p=P
                            )
                            for c in range(CPP):
                                base = (u * CPP + c) * OUT
                                if u == 1 and c == CPP - 1:
                                    for h in range(2):
                                        nc.sync.dma_start(
                                            out=ep[:, base + h * 512 : base + (h + 1) * 512],
                                            in_=src_b[:, c * OUT + h * 512 : c * OUT + (h + 1) * 512],
                                        )
                                else:
                                    nc.sync.dma_start(
                                        out=ep[:, base : base + OUT],
                                        in_=src_b[:, c * OUT : (c + 1) * OUT],
                                    )
                for u in range(2):
                    b = 2 * t + u
                    lasts = b == BS - 1
                    ps = ppool.tile([1, OUT], F32)
                    for q in range(CPP):
                        base = (u * CPP + q) * OUT
                        col = xtr[:, q * BS + b : q * BS + b + 1]
                        if not (lasts and q == CPP - 1):
                            epr = eprpool.tile([P, OUT], F32R, tag="epr")
                            nc.vector.tensor_mul(
                                out=epr[:, :],
                                in0=ep[:, base : base + OUT],
                                in1=sig[:, q * OUT : (q + 1) * OUT],
                            )
                            for nh in range(2):
                                nc.tensor.matmul(
                                    ps[0:1, nh * 512 : (nh + 1) * 512],
                                    col,
                                    epr[:, nh * 512 : (nh + 1) * 512],
                                    start=(q == 0),
                                    stop=(q == CPP - 1),
                                )
                        else:
                            # final chunk of the last sample: o-half pieces
                            for nh in range(2):
                                epr = eprpool.tile([P, OUT], F32R, tag="epr")
                                nc.vector.tensor_mul(
                                    out=epr[:, nh * 512 : (nh + 1) * 512],
                                    in0=ep[:, base + nh * 512 : base + (nh + 1) * 512],
                                    in1=sig[:, q * OUT + nh * 512 : q * OUT + (nh + 1) * 512],
                                )
                                nc.tensor.matmul(
                                    ps[0:1, nh * 512 : (nh + 1) * 512],
                                    col,
                                    epr[:, nh * 512 : (nh + 1) * 512],
                                    start=False,
                                    stop=True,
                                )
                    # evacuate into the 4-row batch tile; store every OB rows
                    ob = b % OB
                    if ob == 0:
                        obat = spool.tile([1, OB * OUT], F32, tag="orow")
                    nc.scalar.copy(obat[0:1, ob * OUT : (ob + 1) * OUT], ps[0:1, :])
                    if ob == OB - 1:
                        nc.gpsimd.dma_start(
                            out=out_d[0:1, (b - OB + 1) * OUT : (b + 1) * OUT],
                            in_=obat,
                        )

    nc.finalize()
    return nc


_NC_CACHE = None


def _get_nc():
    global _NC_CACHE
    if _NC_CACHE is None:
        _NC_CACHE = build_nc()
    return _NC_CACHE


def kernel(x, mu, ro, mu_bias, ro_bias, eps, eps_bias, _trace=False, _tmpdir=None):
    x = np.ascontiguousarray(np.asarray(x, dtype=np.float32))
    mu = np.ascontiguousarray(np.asarray(mu, dtype=np.float32))
    ro = np.ascontiguousarray(np.asarray(ro, dtype=np.float32))
    mu_bias = np.asarray(mu_bias, dtype=np.float32).reshape(1, OUT)
    ro_bias = np.asarray(ro_bias, dtype=np.float32).reshape(1, OUT)
    eps = np.asarray(eps, dtype=np.float32)
    eps_bias = np.ascontiguousarray(np.asarray(eps_bias, dtype=np.float32))

    nc = _get_nc()

    in_maps = []
    for core in range(NCORES):
        g, j = core // ISH, core % ISH
        b0, b1 = g * BS, (g + 1) * BS
        i0, i1 = j * INS, (j + 1) * INS
        # xt[p, c*BS + b] = x[b, i0 + p*CPP + c]  (p-major rows)
        xt = np.ascontiguousarray(
            x[b0:b1, i0:i1].reshape(BS, P, CPP).transpose(1, 2, 0).reshape(P, CPP * BS)
        )
        in_maps.append(
            {
                "eps": np.ascontiguousarray(eps[b0:b1, i0:i1, :]),
                "ro": np.ascontiguousarray(ro[i0:i1, :]),
                "mu": np.ascontiguousarray(mu[i0:i1, :]),
                "xt": xt,
            }
        )

    res = run_bass_kernel_spmd(
        nc, in_maps, core_ids=list(range(NCORES)), trace=_trace, tmpdir=_tmpdir
    )
    # host-side unshard: add i-quarter partials (eps-term rows + x@mu),
    # then the elementwise bias row epilogue.
    bias = eps_bias * np.log1p(np.exp(ro_bias)) + mu_bias  # (B, OUT)
    out = np.empty((B, OUT), dtype=np.float32)
    for g in range(BG):
        acc = res.results[g * ISH]["out"].reshape(BS, OUT) + res.results[g * ISH]["xmu"]
        for j in range(1, ISH):
            acc = (
                acc
                + res.results[g * ISH + j]["out"].reshape(BS, OUT)
                + res.results[g * ISH + j]["xmu"]
            )
        out[g * BS : (g + 1) * BS] = acc + bias[g * BS : (g + 1) * BS]
    if _trace:
        kernel.last_results = res
    return out


# revision 13
# speedup vs baseline: 1.0623x; 1.0623x over previous
"""Bayesian linear layer (per-sample weights) on 8 Trainium2 NeuronCores.

out[b,o] = sum_i x[b,i] * (eps[b,i,o]*softplus(ro)[i,o] + mu[i,o])
           + eps_bias[b,o]*softplus(ro_bias)[o] + mu_bias[o]

Strategy (2D sharding: 4 batch-groups x 2 i-halves per core):
  - Each core handles 32 samples and 512 of the 1024 contraction rows,
    producing a partial sum; the host unshard adds the two i-halves.
    This halves the replicated ro/mu traffic. (ISH=4 variants measured
    WORSE: the consumer pipeline starves the 16 SDMA engines, which run
    at 94% duty in this configuration — they are the binding resource,
    ~410 GB/s effective per NC.)
  - eps rows for one sample ([512, 1024] f32, 2MiB contiguous) stream
    as one [128, 4096] tile on the sync HWDGE ring. ALL loads share
    that ring, sigma params ahead of the stream: SDMA round-robins
    between rings at whole-transfer granularity, so a param load on any
    other ring starves behind queued 2MiB eps transfers.
  - sigma = softplus(ro) via grouped Exp-then-Ln ACT passes.
  - DVE multiplies tiles by softplus(ro), rounding to float32r so
    TensorE consumes them at full rate. (bf16 measured WORSE: f32-in/
    bf16-out TT runs ~20% slower on DVE, and the M=1 matmuls are
    latency-bound, so cheaper rhs does not help.)
  - TensorE reduces over i with M=1 matmuls (lhsT = x column) into a
    [1,1024] PSUM tile per sample; the scalar engine copies PSUM->SBUF
    and stores rows via its ring. x@mu partials ship as a separate
    [32,1024] output; the host unshard adds them plus the (elementwise)
    bias row during the gather, so no per-sample bias matmuls are
    needed.
  - Tail: the final chunk of the last sample is DMA'd and TT'd in
    512-column halves so the matmul/evacuation chain starts before the
    last bytes land.
"""

import numpy as np

import concourse.bass as bass
import concourse.bacc as bacc
import concourse.mybir as mybir
from concourse.tile import TileContext
from concourse.bass_utils import run_bass_kernel_spmd

F32 = mybir.dt.float32
F32R = mybir.dt.float32r
AF = mybir.ActivationFunctionType

B, IN, OUT = 128, 1024, 1024
NCORES = 8
BG = 4                    # batch groups
ISH = NCORES // BG        # i-shards (2)
BS = B // BG              # 32 samples per core
INS = IN // ISH           # 512 contraction rows per core
P = 128
CPP = INS // P            # 4 contraction rows per partition
FREE = CPP * OUT          # 4096 free elems per eps tile (one sample)


def build_nc():
    nc = bacc.Bacc(None, target_bir_lowering=False)

    eps_d = nc.declare_dram_parameter("eps", [BS, INS, OUT], F32, isOutput=False)
    ro_d = nc.declare_dram_parameter("ro", [INS, OUT], F32, isOutput=False)
    mu_d = nc.declare_dram_parameter("mu", [INS, OUT], F32, isOutput=False)
    # xt[p, c*BS + b] = x[b, ishard*512 + c*128 + p]  (host-side layout)
    xt_d = nc.declare_dram_parameter("xt", [P, CPP * BS], F32, isOutput=False)
    out_d = nc.declare_dram_parameter("out", [BS, OUT], F32, isOutput=True)
    xmu_d = nc.declare_dram_parameter("xmu", [BS, OUT], F32, isOutput=True)

    # i_local = c*128 + p: chunk-major, 4KB per-partition DMA runs
    ro_r = ro_d.rearrange("(c p) o -> p c o", p=P)
    mu_r = mu_d.rearrange("(c p) o -> p c o", p=P)

    with TileContext(nc) as tc:
        with (
            tc.tile_pool(name="const", bufs=1) as cpool,
            tc.tile_pool(name="eps", bufs=6) as epool,
            tc.tile_pool(name="epr", bufs=5) as eprpool,
            tc.tile_pool(name="small", bufs=3) as spool,
            tc.tile_pool(name="psmu", bufs=1, space="PSUM") as pmupool,
            tc.tile_pool(name="psum", bufs=3, space="PSUM") as ppool,
        ):
            # ---- sigma params first on the ring, then the eps stream ----
            sig = cpool.tile([P, FREE], F32)
            for h in range(CPP):
                nc.sync.dma_start(
                    out=sig[:, h * OUT : (h + 1) * OUT], in_=ro_r[:, h : h + 1, :]
                )
            xt = cpool.tile([P, CPP * BS], F32)
            nc.sync.dma_start(out=xt, in_=xt_d[:, :])
            for h in range(CPP):
                sl = sig[:, h * OUT : (h + 1) * OUT]
                nc.scalar.activation(sl, sl, AF.Exp)
            for h in range(CPP):
                sl = sig[:, h * OUT : (h + 1) * OUT]
                nc.scalar.activation(sl, sl, AF.Ln, bias=1.0)

            xtr = cpool.tile([P, CPP * BS], F32R)
            nc.vector.tensor_copy(out=xtr, in_=xt)

            # eps[0] ahead of mu: the big stream starts as early as
            # possible; mu is only needed by the x@mu partial.
            ep0 = epool.tile([P, FREE], F32, tag="ep")
            nc.sync.dma_start(
                out=ep0, in_=eps_d[0, :, :].rearrange("(c p) o -> p c o", p=P)
            )

            # ---- x @ mu (partial over this core's i rows) ---------------
            psmu = pmupool.tile([BS, OUT], F32)
            mt = epool.tile([P, FREE], F32, tag="ep")
            nc.sync.dma_start(out=mt, in_=mu_r[:, :, :])
            for c in range(CPP):
                for nh in range(2):
                    nc.tensor.matmul(
                        psmu[:, nh * 512 : (nh + 1) * 512],
                        xt[:, c * BS : (c + 1) * BS],
                        mt[:, c * OUT + nh * 512 : c * OUT + (nh + 1) * 512],
                        start=(c == 0),
                        stop=(c == CPP - 1),
                    )
            oxmu = spool.tile([BS, OUT], F32, tag="oxmu")
            nc.scalar.copy(oxmu, psmu[:, :])
            nc.scalar.dma_start(out=xmu_d[:, :], in_=oxmu)

            # ---- main streaming loop ------------------------------------
            for b in range(BS):
                last = b == BS - 1
                ps = ppool.tile([1, OUT], F32)
                if b == 0:
                    ep = ep0
                else:
                    ep = epool.tile([P, FREE], F32, tag="ep")
                eps_src = eps_d[b, :, :].rearrange("(c p) o -> p c o", p=P)
                if b == 0:
                    pass
                elif not last:
                    nc.sync.dma_start(out=ep, in_=eps_src)
                else:
                    # fine-grained tail: per-chunk DMAs, final chunk split
                    # by o-halves
                    for c in range(CPP):
                        if c < CPP - 1:
                            nc.sync.dma_start(
                                out=ep[:, c * OUT : (c + 1) * OUT],
                                in_=eps_src[:, c : c + 1, :],
                            )
                        else:
                            for h in range(2):
                                nc.sync.dma_start(
                                    out=ep[:, c * OUT + h * 512 : c * OUT + (h + 1) * 512],
                                    in_=eps_src[:, c : c + 1, h * 512 : (h + 1) * 512],
                                )
                if not last:
                    # two TTs per sample, two chunks each
                    for q in range(2):
                        epr = eprpool.tile([P, FREE // 2], F32R, tag="epr")
                        nc.vector.tensor_mul(
                            out=epr[:, :],
                            in0=ep[:, q * 2 * OUT : (q + 1) * 2 * OUT],
                            in1=sig[:, q * 2 * OUT : (q + 1) * 2 * OUT],
                        )
                        for c2 in range(2):
                            c = 2 * q + c2
                            col = xtr[:, c * BS + b : c * BS + b + 1]
                            for nh in range(2):
                                nc.tensor.matmul(
                                    ps[0:1, nh * 512 : (nh + 1) * 512],
                                    col,
                                    epr[:, c2 * OUT + nh * 512 : c2 * OUT + (nh + 1) * 512],
                                    start=(q == 0 and c2 == 0),
                                    stop=(q == 1 and c2 == 1),
                                )
                else:
                    # last sample: chunk-at-a-time, final chunk by o-halves
                    for c in range(CPP):
                        col = xtr[:, c * BS + b : c * BS + b + 1]
                        if c < CPP - 1:
                            epr = eprpool.tile([P, FREE // 2], F32R, tag="epr")
                            nc.vector.tensor_mul(
                                out=epr[:, :OUT],
                                in0=ep[:, c * OUT : (c + 1) * OUT],
                                in1=sig[:, c * OUT : (c + 1) * OUT],
                            )
                            for nh in range(2):
                                nc.tensor.matmul(
                                    ps[0:1, nh * 512 : (nh + 1) * 512],
                                    col,
                                    epr[:, nh * 512 : (nh + 1) * 512],
                                    start=(c == 0),
                                    stop=False,
                                )
                        else:
                            for nh in range(2):
                                epr = eprpool.tile([P, FREE // 2], F32R, tag="epr")
                                nc.vector.tensor_mul(
                                    out=epr[:, nh * 512 : (nh + 1) * 512],
                                    in0=ep[:, c * OUT + nh * 512 : c * OUT + (nh + 1) * 512],
                                    in1=sig[:, c * OUT + nh * 512 : c * OUT + (nh + 1) * 512],
                                )
                                nc.tensor.matmul(
                                    ps[0:1, nh * 512 : (nh + 1) * 512],
                                    col,
                                    epr[:, nh * 512 : (nh + 1) * 512],
                                    start=False,
                                    stop=True,
                                )
                orow = spool.tile([1, OUT], F32, tag="orow")
                nc.scalar.copy(orow, ps[0:1, :])
                nc.scalar.dma_start(out=out_d[b : b + 1, :], in_=orow)

    nc.finalize()
    return nc


_NC_CACHE = None


def _get_nc():
    global _NC_CACHE
    if _NC_CACHE is None:
        _NC_CACHE = build_nc()
    return _NC_CACHE


def kernel(x, mu, ro, mu_bias, ro_bias, eps, eps_bias, _trace=False, _tmpdir=None):
    x = np.ascontiguousarray(np.asarray(x, dtype=np.float32))
    mu = np.ascontiguousarray(np.asarray(mu, dtype=np.float32))
    ro = np.ascontiguousarray(np.asarray(ro, dtype=np.float32))
    mu_bias = np.asarray(mu_bias, dtype=np.float32).reshape(1, OUT)
    ro_bias = np.asarray(ro_bias, dtype=np.float32).reshape(1, OUT)
    eps = np.asarray(eps, dtype=np.float32)
    eps_bias = np.ascontiguousarray(np.asarray(eps_bias, dtype=np.float32))

    nc = _get_nc()

    in_maps = []
    for core in range(NCORES):
        g, j = core // ISH, core % ISH
        b0, b1 = g * BS, (g + 1) * BS
        i0, i1 = j * INS, (j + 1) * INS
        # xt[p, c*BS + b] = x[b, i0 + c*128 + p]
        xt = np.ascontiguousarray(
            x[b0:b1, i0:i1].reshape(BS, CPP, P).transpose(2, 1, 0).reshape(P, CPP * BS)
        )
        in_maps.append(
            {
                "eps": np.ascontiguousarray(eps[b0:b1, i0:i1, :]),
                "ro": np.ascontiguousarray(ro[i0:i1, :]),
                "mu": np.ascontiguousarray(mu[i0:i1, :]),
                "xt": xt,
            }
        )

    res = run_bass_kernel_spmd(
        nc, in_maps, core_ids=list(range(NCORES)), trace=_trace, tmpdir=_tmpdir
    )
    # host-side unshard: add i-half partials (eps-term rows + x@mu), then
    # the elementwise bias row epilogue.
    bias = eps_bias * np.log1p(np.exp(ro_bias)) + mu_bias  # (B, OUT)
    out = np.empty((B, OUT), dtype=np.float32)
    for g in range(BG):
        acc = res.results[g * ISH]["out"] + res.results[g * ISH]["xmu"]
        for j in range(1, ISH):
            acc = acc + res.results[g * ISH + j]["out"] + res.results[g * ISH + j]["xmu"]
        out[g * BS : (g + 1) * BS] = acc + bias[g * BS : (g + 1) * BS]
    if _trace:
        kernel.last_results = res
    return out
